# revision 4
# baseline (speedup 1.0000x reference)
"""Canny NMS filter on 8 Trainium2 NeuronCores — batch-parallel, one image per core.

Per core (img [3,1024,1024] f32 -> out [1024,1024] f32):
  per channel c:
    u  = h5(img_c)/g0                   horizontal 5-tap gaussian (DVE/GPS, shifted APs)
    gx = hdiff(v7a(u))                  PE banded matmuls, hdiff via column-shifted rhs
    gy = h121(v7b(u))                   PE banded matmuls, 3 shifted accumulations
    g += sqrt(gx^2+gy^2+1e-8); s += gx; t += gy
  NMS: axis = sector(s,t) via |t| vs tan(22.5/67.5)*|s|; M = max of the two
  g-neighbors along the axis (vertical shifts via SBUF-SBUF DMA copies);
  out = g*(g>M)*(g>=1); 1px border zeroed.

Row tiling: 9 tiles x 128 rows, stride 120 (4-row halo), so the 7-tap vertical
support + 1-row NMS neighborhood stay inside one tile.
"""
import numpy as np

import concourse.bass as bass
import concourse.mybir as mybir
from concourse.mybir import AluOpType as Op
from concourse.mybir import ActivationFunctionType as AF
from concourse.tile import TileContext
from concourse.bass_utils import run_bass_kernel_spmd

f32 = mybir.dt.float32
u8 = mybir.dt.uint8

H = W = 1024
P = 128
RSTRIDE = 120
NTILES = 9


def split_multiwaits(nc):
    """This container's walrus accepts only one sync-wait per instruction;
    move extra waits onto preceding drain instructions."""
    for fn in nc.m.functions:
        for bb in fn.blocks:
            out = []
            for ins in bb.instructions:
                si = ins.sync_info
                if si is not None and si.on_wait and len(si.on_wait) > 1:
                    waits = list(si.on_wait)
                    for i, w in enumerate(waits[:-1]):
                        out.append(mybir.InstNoOp(
                            name=f"{ins.name}_sw{i}",
                            engine=ins.engine,
                            sync_info=mybir.SyncInfo(on_wait=[w], on_update=[]),
                        ))
                    si.on_wait = [waits[-1]]
                out.append(ins)
            bb.instructions = out


def _gauss5():
    size, std = 5, 1.0
    start = -(size - 1) / 2.0
    c = 1.0 / (std * 2 ** 0.5)
    k = np.linspace(start * c, (start + (size - 1)) * c, size)
    return np.exp(-k ** 2).astype(np.float32)


def _band(taps):
    B = np.zeros((P, P), np.float32)
    for m in range(P):
        for d in range(len(taps)):
            k = m + d - 3
            if 0 <= k < P:
                B[k, m] = taps[d]
    return B


def build_canny():
    g5 = _gauss5()
    g0, g1 = float(g5[0]), float(g5[1])
    r1 = g1 / g0
    r2 = 1.0 / g0
    v7a = np.convolve(g5, np.array([1, 2, 1], np.float32)).astype(np.float32) * g0
    v7b = np.convolve(g5, np.array([1, 0, -1], np.float32)).astype(np.float32) * g0
    T1 = float(np.float32(np.tan(np.deg2rad(22.5))))
    T2 = float(np.float32(np.tan(np.deg2rad(67.5))))

    nc = bass.Bass()
    img = nc.dram_tensor("img", [3, H, W], f32, kind="ExternalInput")
    out = nc.dram_tensor("out", [H, W], f32, kind="ExternalOutput")
    dA = nc.inline_tensor(_band(v7a), "bandA")
    dAn = nc.inline_tensor(_band(-v7a), "bandAn")
    dB1 = nc.inline_tensor(_band(v7b), "bandB1")
    dB2 = nc.inline_tensor(_band(2.0 * v7b), "bandB2")

    with TileContext(nc) as tc:
        with (
            tc.tile_pool(name="wpool", bufs=1) as wp,
            tc.tile_pool(name="sb", bufs=2) as sb,
            tc.tile_pool(name="ps", bufs=2, space="PSUM") as pp,
        ):
            bA = wp.tile([P, P], f32, tag="bA")
            bAn = wp.tile([P, P], f32, tag="bAn")
            bB1 = wp.tile([P, P], f32, tag="bB1")
            bB2 = wp.tile([P, P], f32, tag="bB2")
            nc.sync.dma_start(bA[:], dA[:])
            nc.sync.dma_start(bAn[:], dAn[:])
            nc.sync.dma_start(bB1[:], dB1[:])
            nc.sync.dma_start(bB2[:], dB2[:])
            eps = wp.tile([P, 1], f32, tag="eps")
            nc.vector.memset(eps[:], 1e-8)

            for t in range(NTILES):
                r0 = RSTRIDE * t - 4
                vlo = max(0, -r0)
                vhi = min(P, H - r0)

                g_t = sb.tile([P, W], f32, tag="g")
                s_t = sb.tile([P, W], f32, tag="s")
                t_t = sb.tile([P, W], f32, tag="t")

                for c in range(3):
                    xt = sb.tile([P, W + 8], f32, tag="x")
                    if vlo > 0 or vhi < P:
                        nc.gpsimd.memset(xt[:], 0.0)
                    else:
                        nc.gpsimd.memset(xt[:, 0:3], 0.0)
                        nc.gpsimd.memset(xt[:, W + 3:W + 8], 0.0)
                    nc.sync.dma_start(
                        xt[vlo:vhi, 3:W + 3], img[c, r0 + vlo:r0 + vhi, :])

                    # u = h5(x)/g0 over 1026 cols (image cols -1..1024)
                    a1 = sb.tile([P, W + 2], f32, tag="a1")
                    a2 = sb.tile([P, W + 2], f32, tag="a2")
                    ut = sb.tile([P, W + 2], f32, tag="u")
                    eng1 = nc.gpsimd if c < 2 else nc.vector
                    eng1.tensor_tensor(a1[:], xt[:, 0:W + 2], xt[:, 4:W + 6], Op.add)
                    eng1.tensor_tensor(a2[:], xt[:, 1:W + 3], xt[:, 3:W + 5], Op.add)
                    nc.vector.scalar_tensor_tensor(
                        a2[:], a2[:], r1, a1[:], Op.mult, Op.add)
                    nc.vector.scalar_tensor_tensor(
                        ut[:], xt[:, 2:W + 4], r2, a2[:], Op.mult, Op.add)

                    # gx = A@u[j-1] - A@u[j+1]; gy = B@u[j-1] + 2B@u[j] + B@u[j+1]
                    gx_ps = pp.tile([P, W], f32, tag="gx")
                    gy_ps = pp.tile([P, W], f32, tag="gy")
                    for lo in (0, 512):
                        nc.tensor.matmul(gx_ps[:, lo:lo + 512], bA[:],
                                         ut[:, lo:lo + 512], start=True, stop=False)
                        nc.tensor.matmul(gx_ps[:, lo:lo + 512], bAn[:],
                                         ut[:, lo + 2:lo + 514], start=False, stop=True)
                    for lo in (0, 512):
                        nc.tensor.matmul(gy_ps[:, lo:lo + 512], bB1[:],
                                         ut[:, lo:lo + 512], start=True, stop=False)
                        nc.tensor.matmul(gy_ps[:, lo:lo + 512], bB2[:],
                                         ut[:, lo + 1:lo + 513], start=False, stop=False)
                        nc.tensor.matmul(gy_ps[:, lo:lo + 512], bB1[:],
                                         ut[:, lo + 2:lo + 514], start=False, stop=True)

                    # evacuate gy; square both; mag
                    gye = t_t if c == 0 else sb.tile([P, W], f32, tag="gye")
                    nc.scalar.activation(gye[:], gy_ps[:], AF.Copy)
                    sqx = sb.tile([P, W], f32, tag="sqx")
                    nc.scalar.activation(sqx[:], gx_ps[:], AF.Square)
                    sqy = sb.tile([P, W], f32, tag="sqy")
                    nc.gpsimd.tensor_tensor(sqy[:], gye[:], gye[:], Op.mult)
                    nc.vector.tensor_tensor(sqy[:], sqy[:], sqx[:], Op.add)
                    if c == 0:
                        nc.scalar.activation(g_t[:], sqy[:], AF.Sqrt, bias=eps[:])
                        nc.scalar.activation(s_t[:], gx_ps[:], AF.Copy)
                    else:
                        nc.scalar.activation(sqx[:], sqy[:], AF.Sqrt, bias=eps[:])
                        nc.gpsimd.tensor_tensor(g_t[:], g_t[:], sqx[:], Op.add)
                        nc.vector.tensor_tensor(s_t[:], s_t[:], gx_ps[:], Op.add)
                        nc.gpsimd.tensor_tensor(t_t[:], t_t[:], gye[:], Op.add)

                # ---- NMS ----
                as1 = sb.tile([P, W], f32, tag="as1")
                at = sb.tile([P, W], f32, tag="at")
                as2 = sb.tile([P, W], f32, tag="as2")
                nc.scalar.activation(as1[:], s_t[:], AF.Abs, scale=T1)
                nc.scalar.activation(at[:], t_t[:], AF.Abs)
                nc.scalar.activation(as2[:], s_t[:], AF.Abs, scale=T2)
                m0u = sb.tile([P, W], u8, tag="m0u")
                m2u = sb.tile([P, W], u8, tag="m2u")
                stpu = sb.tile([P, W], u8, tag="stpu")
                nc.vector.tensor_tensor(m0u[:], at[:], as1[:], Op.is_le)
                nc.vector.tensor_tensor(m2u[:], at[:], as2[:], Op.is_gt)
                d_t = sb.tile([P, W], f32, tag="d")
                nc.gpsimd.tensor_tensor(d_t[:], s_t[:], t_t[:], Op.mult)
                nc.vector.tensor_scalar(stpu[:], d_t[:], 0.0, None, Op.is_gt)

                # vertical neighbor shifts via SBUF->SBUF DMA
                gN = sb.tile([P, W], f32, tag="gN")
                gS = sb.tile([P, W], f32, tag="gS")
                nc.sync.dma_start(gN[1:P, :], g_t[0:P - 1, :])
                nc.sync.dma_start(gN[0:1, :], g_t[0:1, :])
                nc.sync.dma_start(gS[0:P - 1, :], g_t[1:P, :])
                nc.sync.dma_start(gS[P - 1:P, :], g_t[P - 1:P, :])

                WW = W - 2
                M0 = sb.tile([P, WW], f32, tag="M0")
                M1 = sb.tile([P, WW], f32, tag="M1")
                M2 = sb.tile([P, WW], f32, tag="M2")
                M3 = sb.tile([P, WW], f32, tag="M3")
                nc.vector.tensor_tensor(M0[:], g_t[:, 0:WW], g_t[:, 2:W], Op.max)
                nc.vector.tensor_tensor(M2[:], gN[:, 1:W - 1], gS[:, 1:W - 1], Op.max)
                nc.vector.tensor_tensor(M1[:], gS[:, 2:W], gN[:, 0:WW], Op.max)
                nc.vector.tensor_tensor(M3[:], gS[:, 0:WW], gN[:, 2:W], Op.max)
                nc.vector.copy_predicated(M3[:], stpu[:, 1:W - 1], M1[:])
                nc.vector.copy_predicated(M3[:], m0u[:, 1:W - 1], M0[:])
                nc.vector.copy_predicated(M3[:], m2u[:, 1:W - 1], M2[:])

                # out = g * (g >= 1) * (g > M)
                pg = sb.tile([P, WW], f32, tag="pg")
                gt_ = sb.tile([P, WW], f32, tag="gt")
                nc.vector.tensor_scalar(pg[:], g_t[:, 1:W - 1], 1.0, None, Op.is_ge)
                nc.vector.tensor_tensor(gt_[:], g_t[:, 1:W - 1], pg[:], Op.mult)
                nc.vector.tensor_tensor(pg[:], gt_[:], M3[:], Op.is_gt)
                nc.vector.tensor_tensor(gt_[:], gt_[:], pg[:], Op.mult)

                olo = max(4, 1 - r0)
                ohi = min(124, (H - 1) - r0)
                nc.sync.dma_start(
                    out[r0 + olo:r0 + ohi, 1:W - 1], gt_[olo:ohi, :])

    split_multiwaits(nc)
    return nc


_CACHE = {}


def kernel(img: np.ndarray) -> np.ndarray:
    img = np.ascontiguousarray(np.asarray(img, dtype=np.float32))
    B = img.shape[0]
    if "nc" not in _CACHE:
        _CACHE["nc"] = build_canny()
    nc = _CACHE["nc"]
    in_maps = [{"img": np.ascontiguousarray(img[i])} for i in range(B)]
    res = run_bass_kernel_spmd(nc, in_maps, core_ids=list(range(B)))
    out = np.stack([res.results[i]["out"] for i in range(B)])[:, None]
    return out.astype(np.float32)


def hw_exec_time_ns(inputs):
    """Run once with NTFF tracing; returns max-core exec time in ns (or None)."""
    img = np.ascontiguousarray(np.asarray(inputs["img"], dtype=np.float32))
    B = img.shape[0]
    if "nc" not in _CACHE:
        _CACHE["nc"] = build_canny()
    in_maps = [{"img": np.ascontiguousarray(img[i])} for i in range(B)]
    res = run_bass_kernel_spmd(
        _CACHE["nc"], in_maps, core_ids=list(range(B)), trace=True)
    return res.exec_time_ns


if __name__ == "__main__":
    rng = np.random.default_rng(0)
    x = rng.uniform(0, 1, (2, 3, H, W)).astype(np.float32)
    y = kernel(x)
    print(y.shape, y.dtype, float(y.max()))


# revision 6
# speedup vs baseline: 1.2176x; 1.2176x over previous
"""Canny NMS filter on 8 Trainium2 NeuronCores — batch-parallel, one image per core.

Per core (img [3,1024,1024] f32 -> out [1024,1024] f32):
  per channel c:
    u  = h5(img_c)/g0                 horizontal 5-tap gaussian (DVE/GPS shifted APs)
    gx = hdiff(v7a(u))                PE banded matmuls, hdiff via column-shifted rhs
    gy = h121(v7b(u))                 PE banded matmuls, 3 shifted accumulations
    g += sqrt(gx^2+gy^2+1e-8); s += gx; t += gy
  NMS: axis = sector(s,t) via |t| vs tan(22.5/67.5)*|s|; M = max of the two
  g-neighbors along the axis (vertical shifts via SBUF-SBUF DMA, merge via
  copy_predicated); out = g*(g>=1)*(g>M); 1px border zeroed.

Row tiling: 9 tiles x 128 rows, stride 120 (4-row halo) so the 7-tap vertical
support + 1-row NMS neighborhood stay inside one tile.
"""
import numpy as np

import concourse.bass as bass
import concourse.mybir as mybir
from concourse.mybir import AluOpType as Op
from concourse.mybir import ActivationFunctionType as AF
from concourse.tile import TileContext
from concourse.bass_utils import run_bass_kernel_spmd

f32 = mybir.dt.float32
u8 = mybir.dt.uint8

H = W = 1024
P = 128
RSTRIDE = 120
NTILES = 9


def split_multiwaits(nc):
    """This container's walrus accepts only one sync-wait per instruction;
    carry extra waits on preceding NoOps."""
    for fn in nc.m.functions:
        for bb in fn.blocks:
            out = []
            for ins in bb.instructions:
                si = ins.sync_info
                if si is not None and si.on_wait and len(si.on_wait) > 1:
                    waits = list(si.on_wait)
                    for i, w in enumerate(waits[:-1]):
                        out.append(mybir.InstNoOp(
                            name=f"{ins.name}_sw{i}",
                            engine=ins.engine,
                            sync_info=mybir.SyncInfo(on_wait=[w], on_update=[]),
                        ))
                    si.on_wait = [waits[-1]]
                out.append(ins)
            bb.instructions = out


def _gauss5():
    size, std = 5, 1.0
    start = -(size - 1) / 2.0
    c = 1.0 / (std * 2 ** 0.5)
    k = np.linspace(start * c, (start + (size - 1)) * c, size)
    return np.exp(-k ** 2).astype(np.float32)


def _band(taps):
    B = np.zeros((P, P), np.float32)
    for m in range(P):
        for d in range(len(taps)):
            k = m + d - 3
            if 0 <= k < P:
                B[k, m] = taps[d]
    return B


def build_canny():
    g5 = _gauss5()
    g0, g1 = float(g5[0]), float(g5[1])
    r1 = g1 / g0
    r2 = 1.0 / g0
    v7a = np.convolve(g5, np.array([1, 2, 1], np.float32)).astype(np.float32) * g0
    v7b = np.convolve(g5, np.array([1, 0, -1], np.float32)).astype(np.float32) * g0
    T1 = float(np.float32(np.tan(np.deg2rad(22.5))))
    T2 = float(np.float32(np.tan(np.deg2rad(67.5))))

    nc = bass.Bass()
    img = nc.dram_tensor("img", [3, H, W], f32, kind="ExternalInput")
    out = nc.dram_tensor("out", [H, W], f32, kind="ExternalOutput")
    dA = nc.inline_tensor(_band(v7a), "bandA")
    dAn = nc.inline_tensor(_band(-v7a), "bandAn")
    dB1 = nc.inline_tensor(_band(v7b), "bandB1")
    dB2 = nc.inline_tensor(_band(2.0 * v7b), "bandB2")

    with TileContext(nc) as tc:
        with (
            tc.tile_pool(name="wpool", bufs=1) as wp,
            tc.tile_pool(name="sb", bufs=2) as sb,
            tc.tile_pool(name="ps", bufs=2, space="PSUM") as pp,
        ):
            bA = wp.tile([P, P], f32, tag="bA")
            bAn = wp.tile([P, P], f32, tag="bAn")
            bB1 = wp.tile([P, P], f32, tag="bB1")
            bB2 = wp.tile([P, P], f32, tag="bB2")
            nc.sync.dma_start(bA[:], dA[:])
            nc.sync.dma_start(bAn[:], dAn[:])
            nc.sync.dma_start(bB1[:], dB1[:])
            nc.sync.dma_start(bB2[:], dB2[:])
            eps = wp.tile([P, 1], f32, tag="eps")
            nc.vector.memset(eps[:], 1e-8)

            for t in range(NTILES):
                r0 = RSTRIDE * t - 4
                vlo = max(0, -r0)
                vhi = min(P, H - r0)

                g_t = sb.tile([P, W], f32, tag="g")
                s_t = sb.tile([P, W], f32, tag="s")
                t_t = sb.tile([P, W], f32, tag="t")

                for c in range(3):
                    xt = sb.tile([P, W + 8], f32, tag="x")
                    if vlo > 0 or vhi < P:
                        nc.gpsimd.memset(xt[:], 0.0)
                    else:
                        nc.gpsimd.memset(xt[:, 0:3], 0.0)
                        nc.gpsimd.memset(xt[:, W + 3:W + 8], 0.0)
                    nc.sync.dma_start(
                        xt[vlo:vhi, 3:W + 3], img[c, r0 + vlo:r0 + vhi, :])

                    # u = h5(x)/g0 over 1026 cols (image cols -1..1024)
                    a1 = sb.tile([P, W + 2], f32, tag="a1")
                    a2 = sb.tile([P, W + 2], f32, tag="a2")
                    ut = sb.tile([P, W + 2], f32, tag="u")
                    nc.gpsimd.tensor_tensor(a1[:], xt[:, 0:W + 2], xt[:, 4:W + 6], Op.add)
                    nc.gpsimd.tensor_tensor(a2[:], xt[:, 1:W + 3], xt[:, 3:W + 5], Op.add)
                    nc.vector.scalar_tensor_tensor(
                        a2[:], a2[:], r1, a1[:], Op.mult, Op.add)
                    nc.vector.scalar_tensor_tensor(
                        ut[:], xt[:, 2:W + 4], r2, a2[:], Op.mult, Op.add)

                    # gx = A@u[j-1] - A@u[j+1]; gy = B@u[j-1] + 2B@u[j] + B@u[j+1]
                    # weight-grouped order: A, An, B2, B1(x2 shifts)
                    gx_ps = pp.tile([P, W], f32, tag="gx")
                    gy_ps = pp.tile([P, W], f32, tag="gy")
                    for lo in (0, 512):
                        nc.tensor.matmul(gx_ps[:, lo:lo + 512], bA[:],
                                         ut[:, lo:lo + 512], start=True, stop=False)
                    for lo in (0, 512):
                        nc.tensor.matmul(gx_ps[:, lo:lo + 512], bAn[:],
                                         ut[:, lo + 2:lo + 514], start=False, stop=True)
                    for lo in (0, 512):
                        nc.tensor.matmul(gy_ps[:, lo:lo + 512], bB2[:],
                                         ut[:, lo + 1:lo + 513], start=True, stop=False)
                    for lo in (0, 512):
                        nc.tensor.matmul(gy_ps[:, lo:lo + 512], bB1[:],
                                         ut[:, lo:lo + 512], start=False, stop=False)
                    for lo in (0, 512):
                        nc.tensor.matmul(gy_ps[:, lo:lo + 512], bB1[:],
                                         ut[:, lo + 2:lo + 514], start=False, stop=True)

                    # evacuate gy; mag = sqrt(gx^2 + gy^2 + 1e-8)
                    gye = t_t if c == 0 else sb.tile([P, W], f32, tag="gye")
                    nc.scalar.activation(gye[:], gy_ps[:], AF.Copy)
                    sqx = sb.tile([P, W], f32, tag="sqx")
                    nc.scalar.activation(sqx[:], gx_ps[:], AF.Square)
                    sqy = sb.tile([P, W], f32, tag="sqy")
                    if c == 0:
                        nc.vector.tensor_tensor(sqy[:], gye[:], gye[:], Op.mult)
                    else:
                        nc.gpsimd.tensor_tensor(sqy[:], gye[:], gye[:], Op.mult)
                    nc.vector.tensor_tensor(sqy[:], sqy[:], sqx[:], Op.add)
                    if c == 0:
                        nc.scalar.activation(g_t[:], sqy[:], AF.Sqrt, bias=eps[:])
                        nc.scalar.activation(s_t[:], gx_ps[:], AF.Copy)
                    else:
                        mg = sb.tile([P, W], f32, tag="mg")
                        nc.scalar.activation(mg[:], sqy[:], AF.Sqrt, bias=eps[:])
                        nc.gpsimd.tensor_tensor(g_t[:], g_t[:], mg[:], Op.add)
                        nc.vector.tensor_tensor(s_t[:], s_t[:], gx_ps[:], Op.add)
                        nc.gpsimd.tensor_tensor(t_t[:], t_t[:], gye[:], Op.add)

                # ---- NMS ----
                as1 = sb.tile([P, W], f32, tag="as1")
                at = sb.tile([P, W], f32, tag="at")
                as2 = sb.tile([P, W], f32, tag="as2")
                nc.scalar.activation(as1[:], s_t[:], AF.Abs, scale=T1)
                nc.scalar.activation(at[:], t_t[:], AF.Abs)
                nc.scalar.activation(as2[:], s_t[:], AF.Abs, scale=T2)
                m0u = sb.tile([P, W], u8, tag="m0u")
                m2u = sb.tile([P, W], u8, tag="m2u")
                stpu = sb.tile([P, W], u8, tag="stpu")
                nc.vector.tensor_tensor(m0u[:], at[:], as1[:], Op.is_le)
                nc.vector.tensor_tensor(m2u[:], at[:], as2[:], Op.is_gt)
                d_t = sb.tile([P, W], f32, tag="d")
                nc.vector.tensor_tensor(d_t[:], s_t[:], t_t[:], Op.mult)
                nc.vector.tensor_scalar(stpu[:], d_t[:], 0.0, None, Op.is_gt)

                # vertical neighbor shifts via SBUF->SBUF DMA (gpsimd queue)
                gN = sb.tile([P, W], f32, tag="gN")
                gS = sb.tile([P, W], f32, tag="gS")
                nc.gpsimd.dma_start(gN[1:P, :], g_t[0:P - 1, :])
                nc.gpsimd.dma_start(gN[0:1, :], g_t[0:1, :])
                nc.gpsimd.dma_start(gS[0:P - 1, :], g_t[1:P, :])
                nc.gpsimd.dma_start(gS[P - 1:P, :], g_t[P - 1:P, :])

                WW = W - 2
                M0 = sb.tile([P, WW], f32, tag="M0")
                M1 = sb.tile([P, WW], f32, tag="M1")
                M2 = sb.tile([P, WW], f32, tag="M2")
                M3 = sb.tile([P, WW], f32, tag="M3")
                nc.vector.tensor_tensor(M0[:], g_t[:, 0:WW], g_t[:, 2:W], Op.max)
                nc.vector.tensor_tensor(M2[:], gN[:, 1:W - 1], gS[:, 1:W - 1], Op.max)
                nc.vector.tensor_tensor(M1[:], gS[:, 2:W], gN[:, 0:WW], Op.max)
                nc.vector.tensor_tensor(M3[:], gS[:, 0:WW], gN[:, 2:W], Op.max)
                nc.vector.copy_predicated(M3[:], stpu[:, 1:W - 1], M1[:])
                nc.vector.copy_predicated(M3[:], m0u[:, 1:W - 1], M0[:])
                nc.vector.copy_predicated(M3[:], m2u[:, 1:W - 1], M2[:])

                # out = g * (g >= 1) * (g > M); full-width aligned store
                pg = sb.tile([P, WW], f32, tag="pg")
                gt_ = sb.tile([P, WW], f32, tag="gt")
                nc.vector.tensor_scalar(pg[:], g_t[:, 1:W - 1], 1.0, None, Op.is_ge)
                nc.vector.tensor_tensor(gt_[:], g_t[:, 1:W - 1], pg[:], Op.mult)
                nc.vector.tensor_tensor(pg[:], gt_[:], M3[:], Op.is_gt)
                outt = sb.tile([P, W], f32, tag="outt")
                nc.gpsimd.memset(outt[:, 0:1], 0.0)
                nc.gpsimd.memset(outt[:, W - 1:W], 0.0)
                nc.vector.tensor_tensor(outt[:, 1:W - 1], gt_[:], pg[:], Op.mult)

                olo = max(4, 1 - r0)
                ohi = min(124, (H - 1) - r0)
                nc.scalar.dma_start(
                    out[r0 + olo:r0 + ohi, :], outt[olo:ohi, :])

    split_multiwaits(nc)
    return nc


_CACHE = {}


def kernel(img: np.ndarray) -> np.ndarray:
    img = np.ascontiguousarray(np.asarray(img, dtype=np.float32))
    B = img.shape[0]
    if "nc" not in _CACHE:
        _CACHE["nc"] = build_canny()
    nc = _CACHE["nc"]
    in_maps = [{"img": np.ascontiguousarray(img[i])} for i in range(B)]
    res = run_bass_kernel_spmd(nc, in_maps, core_ids=list(range(B)))
    out = np.stack([res.results[i]["out"] for i in range(B)])[:, None]
    return out.astype(np.float32)


def hw_exec_time_ns(inputs):
    """Run once with NTFF tracing; returns max-core exec time in ns (or None)."""
    img = np.ascontiguousarray(np.asarray(inputs["img"], dtype=np.float32))
    B = img.shape[0]
    if "nc" not in _CACHE:
        _CACHE["nc"] = build_canny()
    in_maps = [{"img": np.ascontiguousarray(img[i])} for i in range(B)]
    res = run_bass_kernel_spmd(
        _CACHE["nc"], in_maps, core_ids=list(range(B)), trace=True)
    return res.exec_time_ns


if __name__ == "__main__":
    rng = np.random.default_rng(0)
    x = rng.uniform(0, 1, (1, 3, H, W)).astype(np.float32)
    y = kernel(x)
    print(y.shape, y.dtype, float(y.max()), float(np.count_nonzero(y)) / y.size)
